# revision 20
# baseline (speedup 1.0000x reference)
"""Trainium2 Bass kernel for nn_CNNEncoder (hashed n-gram embedding + conv/GLU stack).

Strategy (8 NeuronCores, data-parallel over batch, 2 batches/core):
- Embedding gather via InstDMAGatherAnt (mlp GPSIMD library) spread across 4
  SWDGE queues (4 Q7 cpu pairs generate descriptors in parallel; the serial
  per-descriptor cost of the classic INDIRECT1D path was the baseline's
  bottleneck).  dma_gather idxs are int16, so each order's 50005-row table is
  split into two <=32768-row banks (bank windows of one stacked bf16 table);
  slots are assigned per bank with zero-row padding to each tile's per-bank
  max count.  Tokens are bucket-sorted by word length on the host so padded
  counts stay near the true counts.
- Slot sums on DVE (fp32) via strided [128, E, J] views, scaled by 1/count,
  assembled to [tok, 384] fp32 (e output staging, host unsorts) and scattered
  bf16 (proven [128,1] indirect DMA per tile) into HBM; xbar DMA-transpose
  builds the [384, 2048] conv stripe.
- Conv: weight-norm prep runs once with all 5 layers' bf16 weights resident;
  K-shifted bf16 matmuls accumulate in PSUM; GLU via ACT sigmoid + DVE;
  residual sqrt(0.5) folded into scales, final C^5 folded into layer-5 scales.
- Overlap: batch 1's gathers/reduces are emitted interleaved with batch 0's
  conv layers so DVE FIFO order lets them share the conv window.
"""

import sys

sys.path.insert(0, "/opt/trn_rl_repo")

from contextlib import ExitStack, nullcontext

import numpy as np
import ml_dtypes

import concourse.bass as bass
import concourse.tile as tile
from concourse import bacc, library_config, mybir
from concourse.bass_utils import run_bass_kernel_spmd

B, S, N, E, V, L, KC, LYR = 16, 2048, 3, 128, 50000, 12, 3, 5
W = E * N
C = 0.7071067811865476
NCORES = 8
BPC = B // NCORES           # batches per core
TILES = S // 128            # 16 token tiles per batch
BANK = 32768                # dma_gather int16 idx limit
NW = 2 * N                  # bank windows in the stacked table
GS = 2                      # token tiles per gather group
NG = TILES // GS


def _host_prep(inputs):
    x = np.asarray(inputs["x"]).astype(np.int64)
    ids = np.asarray(inputs["ngram_ids"]).astype(np.int64)
    cnt = np.asarray(inputs["ngram_counts"]).astype(np.int64)
    emb0 = np.asarray(inputs["emb0"]).astype(np.float32)
    tables = np.asarray(inputs["tables"]).astype(np.float32)
    conv_v = np.asarray(inputs["conv_v"]).astype(np.float32)
    conv_g = np.asarray(inputs["conv_g"]).astype(np.float32)
    conv_b = np.asarray(inputs["conv_b"]).astype(np.float32)

    # stacked bank table [6*BANK, 128] bf16: window(n,0)=ids 0..32767 (row 0
    # zero), window(n,1): local0=zero, local k=id 32767+k, emb0 at 17238+x
    tab = np.zeros((NW * BANK, E), dtype=np.float32)
    NB1 = V + 1 - BANK       # 17233 rows in bank 1 (ids 32768..50000)
    for n in range(N):
        tab[(2 * n) * BANK : (2 * n) * BANK + BANK] = tables[n][:BANK]
        w1 = (2 * n + 1) * BANK
        tab[w1 + 1 : w1 + 1 + NB1] = tables[n][BANK:]
        tab[w1 + 1 + NB1 : w1 + 5 + NB1] = emb0[:, n * E : (n + 1) * E]
    tab = tab.astype(ml_dtypes.bfloat16)
    SPECIAL0 = 1 + NB1       # bank-1 local row of emb0[x=0]

    # per (core,batch): sort tokens by total count (== wordlen surrogate)
    special = x < 4                                    # [B, S]
    cnt_eff = np.where(special[..., None], 1, cnt)     # [B, S, 3]
    totc = np.where(special, 1, cnt.sum(-1))           # sort key [B, S]
    perm = np.argsort(totc, axis=1, kind="stable")     # sorted order -> orig pos
    cnt_sorted = np.take_along_axis(cnt_eff, perm[..., None], axis=1)  # [B,S,3]

    # per-slot (bank, local): pad -> (0,0)
    mask = np.arange(L)[None, None, None, :] < cnt_eff[..., None]      # [B,S,3,L]
    bankof = (ids >= BANK) & mask                                      # [B,S,3,L]
    localof = np.where(bankof, ids - (BANK - 1), ids)
    localof = np.where(mask, localof, 0)
    # special tokens: single slot (slot 0) in bank 1 at SPECIAL0+x
    for b in range(B):
        srows = np.nonzero(special[b])[0]
        if len(srows):
            bankof[b, srows] = False
            localof[b, srows] = 0
            bankof[b, srows, :, 0] = True
            localof[b, srows, :, 0] = SPECIAL0 + x[b, srows][:, None]
    is0 = (~bankof) & (localof > 0)                                    # real bank0 slots

    # per-(b, s, n, bank) counts
    cntb = np.zeros((B, S, N, 2), dtype=np.int64)
    cntb[..., 1] = bankof.sum(-1)
    cntb[..., 0] = is0.sum(-1)
    cntb_sorted = np.take_along_axis(
        cntb.reshape(B, S, N * 2), perm[..., None], axis=1
    ).reshape(B, S, N, 2)

    # shared per-tile per-bank max counts (over all batches, tokens in tile)
    Kb = np.zeros((TILES, N, 2), dtype=np.int64)
    for r in range(TILES):
        Kb[r] = cntb_sorted[:, r * 128 : (r + 1) * 128].max(axis=(0, 1))

    # build idx arrays per core in device emission order:
    # for bb, for g, for n, for bank, (cols = sum_{r in g} Kb[r][n][bank])
    def pack(lab, sel, J):
        # lab/sel [128, L]: stable-move selected slots to the front, pad 0
        o = np.argsort(~sel, axis=1, kind="stable")
        s = np.take_along_axis(lab, o, axis=1)[:, :J]
        v = np.take_along_axis(sel, o, axis=1)[:, :J]
        return np.where(v, s, 0)

    per_core = []
    for c in range(NCORES):
        ixcols, rcp, pos = [], [], []
        for bb in range(BPC):
            b = c * BPC + bb
            pm = perm[b]
            for g in range(NG):
                for n in range(N):
                    for bk in range(2):
                        for r in range(g * GS, (g + 1) * GS):
                            J = int(Kb[r][n][bk])
                            if J == 0:
                                continue
                            tokens = pm[r * 128 : (r + 1) * 128]
                            lab = localof[b][tokens, n, :]     # [128, L]
                            sel = (bankof if bk else is0)[b][tokens, n, :]
                            ixcols.append(pack(lab, sel, J))
            for r in range(TILES):
                for n in range(N):
                    rcp.append(1.0 / cnt_sorted[b, r * 128 : (r + 1) * 128, n])
                pos.append(pm[r * 128 : (r + 1) * 128])
        # flatten idx columns to wrapped int16 [128, F]
        allcols = np.concatenate(ixcols, axis=1)          # [128, totC]
        totC = allcols.shape[1]
        flat = allcols.T.reshape(-1)                      # slot i = c*128+p
        wrapped = flat.reshape(-1, 16).T.astype(np.int16) # [16, totC*8]
        ix = np.tile(wrapped, (8, 1))                     # replicate to 128 parts
        per_core.append(
            dict(
                ix=np.ascontiguousarray(ix),
                rcp=np.stack(rcp, axis=1).astype(np.float32),
                pos=np.stack(pos, axis=1).astype(np.int32),
            )
        )

    wv = conv_v.reshape(LYR, 2, 384, 3, 128, KC)
    wv = np.ascontiguousarray(wv.transpose(0, 1, 5, 3, 4, 2))
    cb = np.ascontiguousarray(conv_b.reshape(LYR, 6, 128).transpose(2, 0, 1)).reshape(
        128, LYR * 6
    )
    cg = conv_g.reshape(1, LYR * KC)
    return tab, wv, cg, cb, per_core, Kb, perm


def _build(Kb, repeat=1):
    nc = bacc.Bacc(
        "TRN2", target_bir_lowering=False, debug=False,
        num_swdge_queues=4, dynamic_dma_scratch_size=65536,
    )
    ncols_b = int(Kb.sum())
    FTOT = BPC * ncols_b * 8
    gmax = max(
        int(Kb[g * GS : (g + 1) * GS, n, bk].sum())
        for g in range(NG) for n in range(N) for bk in range(2)
    )
    # idx columns (of 16 int16) per gather group
    gF = [8 * int(Kb[g * GS : (g + 1) * GS].sum()) for g in range(NG)]
    gFmax = max(gF)

    t_tab = nc.dram_tensor("tab", [NW * BANK, E], mybir.dt.bfloat16, kind="ExternalInput")
    t_ix = nc.dram_tensor("ix", [128, FTOT], mybir.dt.int16, kind="ExternalInput")
    t_rcp = nc.dram_tensor("rcp", [128, BPC * TILES * N], mybir.dt.float32, kind="ExternalInput")
    t_pos = nc.dram_tensor("pos", [128, BPC * TILES], mybir.dt.int32, kind="ExternalInput")
    t_wv = nc.dram_tensor("wv", [LYR, 2, KC, 3, 128, 384], mybir.dt.float32, kind="ExternalInput")
    t_cg = nc.dram_tensor("cg", [1, LYR * KC], mybir.dt.float32, kind="ExternalInput")
    t_cb = nc.dram_tensor("cb", [128, LYR * 6], mybir.dt.float32, kind="ExternalInput")
    t_est = [
        nc.dram_tensor(f"e_st{i}", [S, W], mybir.dt.float32, kind="ExternalOutput")
        for i in range(BPC)
    ]
    t_ebst = [
        nc.dram_tensor(f"ebst{i}", [S, W], mybir.dt.bfloat16, kind="Internal")
        for i in range(BPC)
    ]
    t_h = nc.dram_tensor("h_out", [BPC, W, S], mybir.dt.float32, kind="ExternalOutput")

    HW_ = 2112

    with tile.TileContext(nc) as tc, ExitStack() as ctx:
        consts = ctx.enter_context(tc.tile_pool(name="consts", bufs=1))
        wres = ctx.enter_context(tc.tile_pool(name="wres", bufs=1))
        rawp = ctx.enter_context(tc.tile_pool(name="rawp", bufs=1))
        sqp = ctx.enter_context(tc.tile_pool(name="sqp", bufs=1))
        scp = ctx.enter_context(tc.tile_pool(name="scp", bufs=2))
        gd = ctx.enter_context(tc.tile_pool(name="gd", bufs=2))
        ixp = ctx.enter_context(tc.tile_pool(name="ixp", bufs=2))
        redp = ctx.enter_context(tc.tile_pool(name="redp", bufs=3))
        asmp = ctx.enter_context(tc.tile_pool(name="asmp", bufs=2))
        bfp = ctx.enter_context(tc.tile_pool(name="bfp", bufs=2))
        h0p = ctx.enter_context(tc.tile_pool(name="h0p", bufs=2))
        hstr = ctx.enter_context(tc.tile_pool(name="hstr", bufs=2))
        sgp = ctx.enter_context(tc.tile_pool(name="sgp", bufs=3))
        hop = ctx.enter_context(tc.tile_pool(name="hop", bufs=1))
        psc = ctx.enter_context(tc.tile_pool(name="psc", bufs=3, space="PSUM"))
        psm = ctx.enter_context(tc.tile_pool(name="psm", bufs=1, space="PSUM"))

        rcp_t = consts.tile([128, BPC * TILES * N], mybir.dt.float32)
        nc.sync.dma_start(rcp_t[:], t_rcp.ap())
        pos_t = consts.tile([128, BPC * TILES], mybir.dt.int32)
        nc.sync.dma_start(pos_t[:], t_pos.ap())
        cb_t = consts.tile([128, LYR * 6], mybir.dt.float32)
        nc.sync.dma_start(cb_t[:], t_cb.ap())
        cg_t = consts.tile([1, LYR * KC], mybir.dt.float32)
        nc.sync.dma_start(cg_t[:], t_cg.ap())
        ones = consts.tile([128, 128], mybir.dt.float32)
        nc.vector.memset(ones[:], 1.0)

        rep_ctx = tc.For_i(0, repeat, 1) if repeat > 1 else nullcontext()
        ctx.enter_context(rep_ctx)

        # ---------------- weight prep: once, all 5 layers resident ----------
        wT = wres.tile([128, LYR, 2, KC, 3, 384], mybir.dt.bfloat16)
        bae = wres.tile([128, LYR, KC], mybir.dt.float32)
        for l in range(LYR):
            ssq = scp.tile([128, 2, KC], mybir.dt.float32, name=f"q2{l}", tag="ssq2")
            sq = sqp.tile([128, 3, 384], mybir.dt.bfloat16, name=f"s2{l}", tag="sq")
            for h in range(2):
                for k in range(KC):
                    raw = rawp.tile([128, 3, 384], mybir.dt.float32,
                                    name=f"rw{l}_{h}_{k}", tag="raw")
                    nc.sync.dma_start(
                        raw[:], t_wv.ap()[l, h, k].rearrange("c i o -> i c o")
                    )
                    nc.scalar.square(sq[:], raw[:])
                    nc.vector.tensor_reduce(
                        ssq[:, h, k : k + 1], sq[:],
                        axis=mybir.AxisListType.XY, op=mybir.AluOpType.add,
                    )
            ssa = scp.tile([128, KC], mybir.dt.float32, name=f"sa{l}", tag="ssa")
            nc.vector.tensor_add(ssa[:], ssq[:, 0, :], ssq[:, 1, :])
            pnrm = psm.tile([1, KC], mybir.dt.float32, space="PSUM", name=f"pn{l}", tag="pn")
            nc.tensor.matmul(pnrm[:], ones[:, 0:1], ssa[:], start=True, stop=True)
            nrm = scp.tile([1, 8], mybir.dt.float32, name=f"nr{l}", tag="nrm")
            nc.scalar.sqrt(nrm[0:1, 0:KC], pnrm[:])
            nc.vector.reciprocal(nrm[0:1, 3:6], nrm[0:1, 0:KC])
            nc.vector.tensor_mul(
                nrm[0:1, 0:KC], nrm[0:1, 3:6], cg_t[0:1, l * KC : (l + 1) * KC]
            )
            a_extra = C**LYR if l == LYR - 1 else 1.0
            sab = scp.tile([1, 6], mybir.dt.float32, name=f"sb{l}", tag="sab")
            nc.vector.tensor_scalar_mul(sab[0:1, 0:3], nrm[0:1, 0:KC], a_extra)
            nc.vector.tensor_scalar_mul(sab[0:1, 3:6], nrm[0:1, 0:KC], C**l)
            psb = psm.tile([128, 6], mybir.dt.float32, space="PSUM", name=f"pb{l}", tag="pb")
            nc.tensor.matmul(psb[:], ones[0:1, 0:128], sab[0:1, :], start=True, stop=True)
            sbc = scp.tile([128, 6], mybir.dt.float32, name=f"sc{l}", tag="sbc")
            nc.vector.tensor_copy(sbc[:], psb[:])
            for h in range(2):
                for k in range(KC):
                    raw = rawp.tile([128, 3, 384], mybir.dt.float32,
                                    name=f"rx{l}_{h}_{k}", tag="raw")
                    nc.sync.dma_start(
                        raw[:], t_wv.ap()[l, h, k].rearrange("c i o -> i c o")
                    )
                    nc.vector.tensor_scalar_mul(
                        wT[:, l, h, k, :, :], raw[:],
                        sbc[:, h * KC + k : h * KC + k + 1],
                    )
            nc.vector.tensor_scalar_mul(
                bae[:, l, :], cb_t[:, l * 6 : l * 6 + 3], C ** (-l) * a_extra
            )

        # ---------------- embedding (per group emission) ----------------
        qctr = [0]
        foff = [0]

        def emb_group(bb, g):
            ixg = ixp.tile([128, gFmax], mybir.dt.int16, name=f"ix{bb}_{g}", tag="ix")
            nc.sync.dma_start(ixg[:, : gF[g]], t_ix.ap()[:, foff[0] : foff[0] + gF[g]])
            loff = 0
            gts = {}
            for n in range(N):
                for bk in range(2):
                    cw = int(Kb[g * GS : (g + 1) * GS, n, bk].sum())
                    if cw == 0:
                        continue
                    gt = gd.tile([128, gmax, E], mybir.dt.bfloat16,
                                 name=f"g{bb}_{g}_{n}_{bk}", tag="gd")
                    ni = 128 * cw
                    nc.gpsimd.dma_gather(
                        out_ap=gt[:, :cw, :],
                        in_ap=t_tab.ap()[(2 * n + bk) * BANK : (2 * n + bk + 1) * BANK, :],
                        idxs_ap=ixg[:, loff : loff + ni // 16],
                        num_idxs=ni, num_idxs_reg=ni, elem_size=E,
                        single_packet=False, queue_num=qctr[0] % 4,
                    )
                    qctr[0] += 1
                    loff += ni // 16
                    gts[(n, bk)] = gt
            foff[0] += gF[g]
            asm = asmp.tile([128, GS, W], mybir.dt.float32, name=f"a{bb}_{g}", tag="asm")
            for n in range(N):
                base = {0: 0, 1: 0}
                for ri, r in enumerate(range(g * GS, (g + 1) * GS)):
                    col = (bb * TILES + r) * N + n
                    parts = []
                    for bk in range(2):
                        J = int(Kb[r][n][bk])
                        if J > 0:
                            parts.append((bk, base[bk], J))
                            base[bk] += J
                    outs = []
                    for pi, (bk, b0, J) in enumerate(parts):
                        gt = gts[(n, bk)]
                        if J == 1:
                            outs.append(gt[:, b0, :])
                        else:
                            red = redp.tile([128, E], mybir.dt.float32,
                                            name=f"rd{bb}_{g}_{n}_{ri}_{bk}", tag="red")
                            gv = gt[:, b0 : b0 + J, :].rearrange("p j e -> p e j")
                            nc.vector.tensor_reduce(
                                red[:], gv, axis=mybir.AxisListType.X,
                                op=mybir.AluOpType.add,
                            )
                            outs.append(red[:])
                    dst = asm[:, ri, n * E : (n + 1) * E]
                    if len(outs) == 2:
                        ssum = redp.tile([128, E], mybir.dt.float32,
                                         name=f"sm{bb}_{g}_{n}_{ri}", tag="red")
                        nc.vector.tensor_add(ssum[:], outs[0], outs[1])
                        nc.vector.tensor_scalar_mul(dst, ssum[:], rcp_t[:, col : col + 1])
                    else:
                        nc.vector.tensor_scalar_mul(dst, outs[0], rcp_t[:, col : col + 1])
            bfw = bfp.tile([128, GS, W], mybir.dt.bfloat16, name=f"b{bb}_{g}", tag="bf")
            nc.vector.tensor_copy(bfw[:], asm[:])
            for ri, r in enumerate(range(g * GS, (g + 1) * GS)):
                nc.sync.dma_start(
                    t_est[bb].ap()[r * 128 : (r + 1) * 128, :], asm[:, ri, :]
                )
                pcol = bb * TILES + r
                nc.gpsimd.indirect_dma_start(
                    out=t_ebst[bb].ap(),
                    out_offset=bass.IndirectOffsetOnAxis(
                        ap=pos_t[:, pcol : pcol + 1], axis=0
                    ),
                    in_=bfw[:, ri, :],
                    in_offset=None,
                )

        def emit_stripe(bb):
            h0 = h0p.tile([128, N, HW_], mybir.dt.bfloat16, name=f"h0_{bb}", tag="h0")
            nc.vector.memset(h0[:, :, 31:32], 0.0)
            nc.vector.memset(h0[:, :, 2080:2081], 0.0)
            for n in range(N):
                nc.sync.dma_start(
                    h0[:, n, 32:2080],
                    t_ebst[bb].ap()[:, n * E : (n + 1) * E],
                    transpose=True,
                )
            return h0

        def conv_layer(bb, l, hcur):
            hnext = (
                hstr.tile([128, N, HW_], mybir.dt.bfloat16, name=f"h{bb}_{l + 1}", tag="hs")
                if l < LYR - 1
                else None
            )
            if hnext is not None:
                nc.vector.memset(hnext[:, :, 31:32], 0.0)
                nc.vector.memset(hnext[:, :, 2080:2081], 0.0)
            for pj in range(3):
                for nt in range(4):
                    ps_a = psc.tile([128, 512], mybir.dt.float32, space="PSUM",
                                    name=f"pa{bb}{l}{pj}{nt}", tag="psa")
                    ps_b = psc.tile([128, 512], mybir.dt.float32, space="PSUM",
                                    name=f"pq{bb}{l}{pj}{nt}", tag="psb")
                    for ci in range(3):
                        for k in range(KC):
                            rhs = hcur[:, ci, 32 + nt * 512 + k - 1 : 32 + nt * 512 + k + 511]
                            st = ci == 0 and k == 0
                            sp = ci == 2 and k == KC - 1
                            nc.tensor.matmul(
                                ps_a[:], wT[:, l, 0, k, ci, pj * 128 : (pj + 1) * 128],
                                rhs, start=st, stop=sp,
                            )
                            nc.tensor.matmul(
                                ps_b[:], wT[:, l, 1, k, ci, pj * 128 : (pj + 1) * 128],
                                rhs, start=st, stop=sp,
                            )
                    sig = sgp.tile([128, 512], mybir.dt.bfloat16,
                                   name=f"sg{bb}{l}{pj}{nt}", tag="sig")
                    nc.scalar.activation(
                        sig[:], ps_b[:], mybir.ActivationFunctionType.Sigmoid,
                        bias=cb_t[:, l * 6 + 3 + pj : l * 6 + 4 + pj], scale=1.0,
                    )
                    if hnext is not None:
                        glu = sgp.tile([128, 512], mybir.dt.bfloat16,
                                       name=f"gl{bb}{l}{pj}{nt}", tag="glu")
                        nc.vector.scalar_tensor_tensor(
                            glu[:], ps_a[:], bae[:, l, pj : pj + 1], sig[:],
                            op0=mybir.AluOpType.add, op1=mybir.AluOpType.mult,
                        )
                        nc.vector.tensor_add(
                            hnext[:, pj, 32 + nt * 512 : 32 + (nt + 1) * 512],
                            glu[:],
                            hcur[:, pj, 32 + nt * 512 : 32 + (nt + 1) * 512],
                        )
                    else:
                        glu = sgp.tile([128, 512], mybir.dt.bfloat16,
                                       name=f"gl{bb}{l}{pj}{nt}", tag="glu")
                        nc.vector.scalar_tensor_tensor(
                            glu[:], ps_a[:], bae[:, l, pj : pj + 1], sig[:],
                            op0=mybir.AluOpType.add, op1=mybir.AluOpType.mult,
                        )
                        hs = hop.tile([128, 512], mybir.dt.float32,
                                      name=f"hs{bb}{pj}{nt}", tag="hsc")
                        nc.vector.scalar_tensor_tensor(
                            hs[:], hcur[:, pj, 32 + nt * 512 : 32 + (nt + 1) * 512],
                            C**LYR, glu[:],
                            op0=mybir.AluOpType.mult, op1=mybir.AluOpType.add,
                        )
                        nc.sync.dma_start(
                            t_h.ap()[bb][pj * 128 : (pj + 1) * 128,
                                         nt * 512 : (nt + 1) * 512],
                            hs[:],
                        )
            return hnext if hnext is not None else hcur

        # emission: E(b0); T(b0); conv(b0) layers interleaved with E(b1)
        for g in range(NG):
            emb_group(0, g)
        h_b0 = emit_stripe(0)
        for l in range(LYR):
            h_b0 = conv_layer(0, l, h_b0)
            if l < 4:
                for g in range(2 * l, 2 * l + 2):
                    emb_group(1, g)
        h_b1 = emit_stripe(1)
        for l in range(LYR):
            h_b1 = conv_layer(1, l, h_b1)
    nc.compile()
    return nc


_CACHE = {}


def _run(inputs, trace=False, repeat=1):
    tab, wv, cg, cb, per_core, Kb, perm = _host_prep(inputs)
    key = (Kb.tobytes(), repeat)
    if key not in _CACHE:
        _CACHE[key] = _build(Kb, repeat=repeat)
    nc = _CACHE[key]
    in_maps = [
        dict(tab=tab, ix=pc["ix"], rcp=pc["rcp"], pos=pc["pos"], wv=wv, cg=cg, cb=cb)
        for pc in per_core
    ]
    res = run_bass_kernel_spmd(nc, in_maps, core_ids=list(range(NCORES)), trace=trace)
    h = np.concatenate([r["h_out"] for r in res.results], axis=0)
    e = np.empty((B, W, S), dtype=np.float32)
    for c in range(NCORES):
        for i in range(BPC):
            b = c * BPC + i
            es = np.empty((S, W), dtype=np.float32)
            es[perm[b]] = res.results[c][f"e_st{i}"]
            e[b] = es.T
    return (h.astype(np.float32), np.ascontiguousarray(e).astype(np.float32)), res


def kernel(**inputs):
    out, _ = _run(inputs)
    return out
